# revision 5
# baseline (speedup 1.0000x reference)
"""Trainium2 Bass kernel for EpsModel.

Math: for each 2x2 batch matrix X (B of them) and a fixed 2x2 A = A0inv:
    T  = X @ A
    ft = A @ (I - T) @ (I + T@T) = A @ (I - T + T^2 - T^3)
Cayley-Hamilton for 2x2: T^2 = s*T - d*I  (s = tr T, d = det T), so
    I - T + T^2 - T^3 = alpha*I + beta*T
    alpha = 1 + d*(s - 1)
    beta  = -1 + s - s^2 + d
and  ft = A @ (alpha*I + beta*T)  with  d = detA * (x0*x3 - x1*x2),
s = t0 + t3.  Pure streaming elementwise pipeline on DVE + ACT in the
natural interleaved layout (quad = [x0,x1,x2,x3] contiguous).

Sharding: data-parallel over the leading batch dim across 8 cores.
"""

import sys

if "/opt/trn_rl_repo" not in sys.path:
    sys.path.insert(0, "/opt/trn_rl_repo")

import numpy as np

B_TOTAL = 8388608
N_CORES = 8
BPC = B_TOTAL // N_CORES        # batch elements per core

Q = 1024                        # quads (batch elems) per partition per chunk
CHUNK = 128 * Q
N_CHUNKS = BPC // CHUNK
FREE = 4 * Q

assert BPC % CHUNK == 0


def build_nc(a, b, c, d, n_chunks=N_CHUNKS, q=Q, reps=1):
    """Build the per-core Bass kernel with A entries folded in as immediates.

    reps>1 wraps the whole pipeline in a For_i hardware loop re-running the
    same work (used only for device-side timing amortization in test.py).
    """
    import concourse.mybir as mybir
    import concourse.bacc as bacc
    from concourse import tile
    from contextlib import ExitStack

    f32 = mybir.dt.float32
    Alu = mybir.AluOpType
    Act = mybir.ActivationFunctionType

    a = float(a); b = float(b); c = float(c); d = float(d)
    detA = a * d - b * c
    free = 4 * q

    nc = bacc.Bacc("TRN2", target_bir_lowering=False, debug=False)
    x_d = nc.dram_tensor("x", [n_chunks, 128, free], f32, kind="ExternalInput").ap()
    o_d = nc.dram_tensor("out", [n_chunks, 128, free], f32, kind="ExternalOutput").ap()

    def pairs(ap):
        return ap.rearrange("p (q f) -> p q f", f=2)

    def quads(ap):
        return ap.rearrange("p (q f) -> p q f", f=4)

    with tile.TileContext(nc) as tc, ExitStack() as ctx:
        io = ctx.enter_context(tc.tile_pool(name="io", bufs=2))
        pp = ctx.enter_context(tc.tile_pool(name="pp", bufs=2))
        qq = ctx.enter_context(tc.tile_pool(name="qq", bufs=2))

        rep_ctx = tc.For_i(0, reps, 1) if reps > 1 else None
        if rep_ctx is not None:
            rep_ctx.__enter__()

        for ci in range(n_chunks):
            x = io.tile([128, free], f32, tag="x", name=f"x{ci}")
            nc.sync.dma_start(x, x_d[ci])
            xq = quads(x)
            xp = pairs(x)
            xe = xp[:, :, 0]                     # [x0, x2] stream  [128, 2q]
            xo = xp[:, :, 1]                     # [x1, x3] stream
            x0 = xq[:, :, 0]; x1 = xq[:, :, 1]   # [128, q] each
            x2 = xq[:, :, 2]; x3 = xq[:, :, 3]

            # ---- T = X @ A as even/odd pair streams (ACT scale + DVE fused MAC)
            # te = [t0, t2] = a*xe + c*xo ; to = [t1, t3] = b*xe + d*xo
            te = pp.tile([128, 2 * q], f32, tag="te", name=f"te{ci}")
            nc.scalar.activation(te, xo, Act.Copy, bias=0.0, scale=c)
            nc.vector.scalar_tensor_tensor(te, xe, a, te, Alu.mult, Alu.add)
            to = pp.tile([128, 2 * q], f32, tag="to", name=f"to{ci}")
            nc.scalar.activation(to, xo, Act.Copy, bias=0.0, scale=d)
            nc.vector.scalar_tensor_tensor(to, xe, b, to, Alu.mult, Alu.add)

            tep = pairs(te); top = pairs(to)
            t0 = tep[:, :, 0]; t2 = tep[:, :, 1]
            t1 = top[:, :, 0]; t3 = top[:, :, 1]

            # ---- scalar streams ----
            s = qq.tile([128, q], f32, tag="s", name=f"s{ci}")
            nc.vector.tensor_tensor(s, t0, t3, Alu.add)

            m1 = qq.tile([128, q], f32, tag="qa", name=f"m1_{ci}")
            nc.vector.tensor_tensor(m1, x0, x3, Alu.mult)
            m2 = qq.tile([128, q], f32, tag="qb", name=f"m2_{ci}")
            nc.vector.tensor_tensor(m2, x1, x2, Alu.mult)
            dx = qq.tile([128, q], f32, tag="dx", name=f"dx{ci}")
            nc.vector.tensor_tensor(dx, m1, m2, Alu.subtract)

            sm1 = qq.tile([128, q], f32, tag="qa", name=f"sm1_{ci}")
            nc.scalar.activation(sm1, s, Act.Copy, bias=-1.0, scale=1.0)

            dsm = qq.tile([128, q], f32, tag="qb", name=f"dsm{ci}")
            nc.vector.tensor_tensor(dsm, dx, sm1, Alu.mult)
            alpha = qq.tile([128, q], f32, tag="alpha", name=f"al{ci}")
            nc.scalar.activation(alpha, dsm, Act.Copy, bias=1.0, scale=detA)

            v = qq.tile([128, q], f32, tag="qb", name=f"v{ci}")
            nc.vector.tensor_tensor(v, s, sm1, Alu.mult)          # s^2 - s
            r = qq.tile([128, q], f32, tag="qa", name=f"r{ci}")
            nc.vector.scalar_tensor_tensor(r, dx, detA, v, Alu.mult, Alu.subtract)
            beta = qq.tile([128, q], f32, tag="beta", name=f"be{ci}")
            nc.scalar.activation(beta, r, Act.Copy, bias=-1.0, scale=1.0)

            # ---- G = alpha*I + beta*T, computed in place over T ----
            nc.vector.tensor_tensor(t0, beta, t0, Alu.mult)
            nc.vector.tensor_tensor(t1, beta, t1, Alu.mult)
            nc.vector.tensor_tensor(t2, beta, t2, Alu.mult)
            nc.vector.tensor_tensor(t3, beta, t3, Alu.mult)
            nc.vector.tensor_tensor(t0, t0, alpha, Alu.add)       # g0
            nc.vector.tensor_tensor(t3, t3, alpha, Alu.add)       # g3
            g0, g1, g2, g3 = t0, t1, t2, t3

            # ---- ft = A @ G ----
            # f0 = a*g0 + b*g2 ; f1 = a*g1 + b*g3
            # f2 = c*g0 + d*g2 ; f3 = c*g1 + d*g3
            out = io.tile([128, free], f32, tag="o", name=f"o{ci}")
            oq = quads(out)
            f0 = oq[:, :, 0]; f1 = oq[:, :, 1]
            f2 = oq[:, :, 2]; f3 = oq[:, :, 3]

            if abs(b) >= abs(d):
                # scale g2/g3 by b; f0/f1 fused; f2/f3 use ratio d/b
                rt = d / b if b != 0.0 else 0.0
                nc.scalar.activation(g2, g2, Act.Copy, bias=0.0, scale=b)
                nc.vector.scalar_tensor_tensor(f0, g0, a, g2, Alu.mult, Alu.add)
                nc.scalar.activation(g0, g0, Act.Copy, bias=0.0, scale=c)
                nc.vector.scalar_tensor_tensor(f2, g2, rt, g0, Alu.mult, Alu.add)
                nc.scalar.activation(g3, g3, Act.Copy, bias=0.0, scale=b)
                nc.vector.scalar_tensor_tensor(f1, g1, a, g3, Alu.mult, Alu.add)
                nc.scalar.activation(g1, g1, Act.Copy, bias=0.0, scale=c)
                nc.vector.scalar_tensor_tensor(f3, g3, rt, g1, Alu.mult, Alu.add)
            else:
                # scale g2/g3 by d; f2/f3 fused; f0/f1 use ratio b/d
                rt = b / d
                nc.scalar.activation(g2, g2, Act.Copy, bias=0.0, scale=d)
                nc.vector.scalar_tensor_tensor(f2, g0, c, g2, Alu.mult, Alu.add)
                nc.scalar.activation(g0, g0, Act.Copy, bias=0.0, scale=a)
                nc.vector.scalar_tensor_tensor(f0, g2, rt, g0, Alu.mult, Alu.add)
                nc.scalar.activation(g3, g3, Act.Copy, bias=0.0, scale=d)
                nc.vector.scalar_tensor_tensor(f3, g1, c, g3, Alu.mult, Alu.add)
                nc.scalar.activation(g1, g1, Act.Copy, bias=0.0, scale=a)
                nc.vector.scalar_tensor_tensor(f1, g3, rt, g1, Alu.mult, Alu.add)

            nc.sync.dma_start(o_d[ci], out)

        if rep_ctx is not None:
            rep_ctx.__exit__(None, None, None)

    nc.compile()
    return nc


def kernel(x, A0inv):
    x = np.ascontiguousarray(np.asarray(x, dtype=np.float32))
    A = np.asarray(A0inv, dtype=np.float32)
    a, b = float(A[0, 0]), float(A[0, 1])
    c, d = float(A[1, 0]), float(A[1, 1])

    from concourse.bass_utils import run_bass_kernel_spmd

    nc = build_nc(a, b, c, d)

    shards = x.reshape(N_CORES, N_CHUNKS, 128, FREE)
    in_maps = [{"x": shards[i]} for i in range(N_CORES)]
    res = run_bass_kernel_spmd(nc, in_maps, list(range(N_CORES)))
    out = np.concatenate(
        [r["out"].reshape(BPC, 2, 2) for r in res.results], axis=0
    )
    return out.astype(np.float32, copy=False)
